# revision 19
# baseline (speedup 1.0000x reference)
"""Low-rank layer y = (U^T V) @ x computed as y = U^T @ (V @ x).

Full problem: x [8192, 4096] f32, U/V [8, 8192] f32, y [8192, 4096] f32.
Sharding: batch (columns of x) split across 8 NeuronCores, 512 per core.

Per core, the 512-column shard is processed as two 256-column halves so
the store stream of half A overlaps the load stream of half B: loads go
through SWDGE (gpsimd, casting f32->bf16 inline), stores through HWDGE
(sync) — separate descriptor rings, so the SDMA engines round-robin
between them.  PE runs in bf16 (fp32 matmuls are 4x slower and were the
bottleneck); fp32 PSUM accumulation keeps rel err ~4e-3 vs the 2e-2 gate.
Memory-bound: 32 MiB HBM traffic per core -> ~94 us floor at 358 GB/s.
"""

import numpy as np
import ml_dtypes

L = 8192
RANK = 8
BATCH = 4096
NCORES = 8
BS = BATCH // NCORES  # 512 batch columns per core
P = 128               # SBUF partitions
NCHUNK = L // P       # 64 row-chunks of 128
NB = 2                # batch halves per core (overlap stores_a with loads_b)
CB = BS // NB         # 256 columns per half
# Per-half load groups (in L-chunks): a small tail group shrinks the PE
# lag between last-load-end and the half's T completion.
XGROUPS = [16, 16, 16, 8, 4, 4]
# Per-half store stages (in L-chunks), ascending: a small first stage gets
# the store ring going ~1.3 us sooner after T is ready (the ring is
# saturated after that, so later stages can be big), and fewer stores
# shrink the end-of-kernel HWDGE lane-drain chain.
YGROUPS = [4, 12, 16, 16, 16]
assert sum(XGROUPS) == NCHUNK
assert sum(YGROUPS) == NCHUNK  # 64 L-chunks of y per half

_NC = None  # cached compiled Bass module


def _body(tc, nc, x, vt, u, y, mybir):
    from contextlib import ExitStack

    f32 = mybir.dt.float32
    bf16 = mybir.dt.bfloat16
    x3 = x.rearrange("(n p) b -> p n b", p=P)   # [128, 64, 512] view of DRAM
    y3 = y.rearrange("(n p) b -> p n b", p=P)

    # Wait-topology discipline (walrus encodes ~1 sync wait per instruction):
    #  - load DMAs write distinct SBUF slots (no reuse waits);
    #  - the first PE instruction of each weight phase is a dummy matmul
    #    absorbing that weight tensor's DMA wait;
    #  - PSUM->SBUF copies alternate ACT/DVE (multi-sem waits legal there).
    with ExitStack() as ctx:
        const = ctx.enter_context(tc.tile_pool(name="const", bufs=1))
        # Slot count is per-tag: all max-size groups share tag "xtb", the
        # rest "xts", each with enough bufs that no slot is ever reused
        # (keeps the load DMAs wait-free).
        nbig = NB * sum(1 for g in XGROUPS if g == max(XGROUPS))
        nsmall = NB * len(XGROUPS) - nbig
        xpool = ctx.enter_context(tc.tile_pool(name="xbuf", bufs=1))
        tpsum = ctx.enter_context(tc.tile_pool(name="tpsum", bufs=1, space="PSUM"))
        ypsum = ctx.enter_context(tc.tile_pool(name="ypsum", bufs=4, space="PSUM"))
        ystage = ctx.enter_context(tc.tile_pool(name="ystage", bufs=4))

        # Issue ALL x load DMAs first (Pool-engine FIFO: they have no sync
        # waits, so the SWDGE ring streams both halves back to back while
        # the HWDGE store ring joins in as stages become ready).
        xts = []  # [half][group] -> SBUF tile
        for h in range(NB):
            tiles = []
            n0 = 0
            for g, xg in enumerate(XGROUPS):
                big = xg == max(XGROUPS)
                xt = xpool.tile([P, xg * CB], bf16,
                                tag="xtb" if big else "xts",
                                bufs=nbig if big else nsmall)
                nc.gpsimd.dma_start(
                    xt[:].rearrange("p (n b) -> p n b", b=CB),
                    x3[:, n0:n0 + xg, h * CB:(h + 1) * CB],
                )
                tiles.append(xt)
                n0 += xg
            xts.append(tiles)

        # Tiny replicated operands (already bf16 from the host), on the
        # HWDGE ring so they ride alongside the SWDGE x stream.
        vt_sb = const.tile([P, NCHUNK * RANK], bf16)  # vt[p, n*8+r] = V[r, n*128+p]
        nc.sync.dma_start(vt_sb[:], vt[:])
        u_sb = const.tile([RANK, L], bf16)
        nc.sync.dma_start(u_sb[:], u[:])

        # Dummy matmul reading ONLY vt_sb: absorbs the vt DMA wait so the
        # first real matmul carries a single sync wait.
        warm1 = tpsum.tile([RANK, RANK], f32, tag="warm1")
        nc.tensor.matmul(warm1[:], vt_sb[:, 0:RANK], vt_sb[:, 0:RANK],
                         start=True, stop=True)

        warmed2 = False
        for h in range(NB):
            # Phase 1 for this half: T_h = V @ x_h, accumulated in PSUM.
            t_ps = tpsum.tile([RANK, BS], f32, tag=f"t{h}")
            n0 = 0
            for g, xg in enumerate(XGROUPS):
                xt = xts[h][g]
                for c in range(xg):
                    n = n0 + c
                    nc.tensor.matmul(
                        t_ps[:, 0:CB],
                        vt_sb[:, n * RANK:(n + 1) * RANK],  # lhsT [128, 8] bf16
                        xt[:, c * CB:(c + 1) * CB],         # rhs  [128, 256] bf16
                        start=(n == 0),
                        stop=(n == NCHUNK - 1),
                    )
                n0 += xg
            # T to SBUF as bf16 (rhs of the expand matmuls), on DVE.
            t_sb = const.tile([RANK, CB], bf16, tag=f"tsb{h}")
            nc.vector.tensor_copy(t_sb[:], t_ps[:, 0:CB])

            if not warmed2:
                # Absorb the u DMA wait on a dummy matmul so the first y
                # matmul waits only on the t_sb copy.
                warm2 = tpsum.tile([P, RANK], f32, tag="warm2")
                nc.tensor.matmul(warm2[:], u_sb[:, 0:P], u_sb[:, 0:RANK],
                                 start=True, stop=True)
                warmed2 = True

            # Phase 2 for this half: y chunk = U_chunk^T @ T_h, staged
            # through SBUF, YGROUPS[d] chunks per HWDGE store.
            m0 = 0
            for d, yg in enumerate(YGROUPS):
                stage = ystage.tile([P, yg * CB], f32,
                                    tag=f"ys{yg}", bufs=3 if yg == 16 else 2)
                for c in range(yg):
                    n = m0 + c
                    y_ps = ypsum.tile([P, BS], f32, tag="yp")
                    nc.tensor.matmul(
                        y_ps[:, 0:CB],
                        u_sb[:, n * P:(n + 1) * P],  # lhsT [8, 128] bf16
                        t_sb[:],                     # rhs  [8, 256] bf16
                        start=True,
                        stop=True,
                    )
                    if c % 2 == 0:
                        nc.scalar.copy(stage[:, c * CB:(c + 1) * CB], y_ps[:, 0:CB])
                    else:
                        nc.vector.tensor_copy(stage[:, c * CB:(c + 1) * CB], y_ps[:, 0:CB])
                nc.sync.dma_start(
                    y3[:, m0:m0 + yg, h * CB:(h + 1) * CB],
                    stage[:].rearrange("p (n b) -> p n b", b=CB),
                )
                m0 += yg


def build_bass():
    import concourse.mybir as mybir
    import concourse.tile as tile
    from concourse import bacc

    # Bacc (not raw Bass): its compile() runs generate_event_semaphores(),
    # which splits multi-sem waits into the 1-wait-per-instruction form the
    # TRN2 ISA requires.
    nc = bacc.Bacc("TRN2", target_bir_lowering=False, debug=False)
    x = nc.dram_tensor("x", [L, BS], mybir.dt.float32, kind="ExternalInput").ap()
    vt = nc.dram_tensor("vt", [P, NCHUNK * RANK], mybir.dt.bfloat16, kind="ExternalInput").ap()
    u = nc.dram_tensor("u", [RANK, L], mybir.dt.bfloat16, kind="ExternalInput").ap()
    y = nc.dram_tensor("y", [L, BS], mybir.dt.float32, kind="ExternalOutput").ap()

    with tile.TileContext(nc) as tc:
        _body(tc, nc, x, vt, u, y, mybir)
    nc.compile()
    return nc


def _get_nc():
    global _NC
    if _NC is None:
        _NC = build_bass()
    return _NC


def make_in_maps(inputs, U, V):
    x = np.asarray(inputs, dtype=np.float32)
    U = np.asarray(U, dtype=np.float32)
    V = np.asarray(V, dtype=np.float32)
    u_bf = np.ascontiguousarray(U.astype(ml_dtypes.bfloat16))
    # vt[p, n*RANK + r] = V[r, n*128 + p]
    vt = np.ascontiguousarray(
        V.reshape(RANK, NCHUNK, P).transpose(2, 1, 0).reshape(P, NCHUNK * RANK)
        .astype(ml_dtypes.bfloat16)
    )
    in_maps = []
    for c in range(NCORES):
        xs = np.ascontiguousarray(x[:, c * BS:(c + 1) * BS])
        in_maps.append({"x": xs, "vt": vt, "u": u_bf})
    return in_maps


def kernel(inputs, U, V):
    from concourse import bass_utils

    nc = _get_nc()
    in_maps = make_in_maps(inputs, U, V)
    res = bass_utils.run_bass_kernel_spmd(nc, in_maps, core_ids=list(range(NCORES)))
    return np.concatenate([res.results[c]["y"] for c in range(NCORES)], axis=1)


# revision 22
# speedup vs baseline: 1.1510x; 1.1510x over previous
"""Low-rank layer y = (U^T V) @ x computed as y = U^T @ (V @ x).

Full problem: x [8192, 4096] f32, U/V [8, 8192] f32, y [8192, 4096] f32.
Sharding: batch (columns of x) split across 8 NeuronCores, 512 per core.

Per core, the 512-column shard is processed as two 256-column halves so
the store stream of half A overlaps the load stream of half B: loads go
through SWDGE (gpsimd, casting f32->bf16 inline), stores through HWDGE
(sync) — separate descriptor rings, so the SDMA engines round-robin
between them.  PE runs in bf16 (fp32 matmuls are 4x slower and were the
bottleneck); fp32 PSUM accumulation keeps rel err ~4e-3 vs the 2e-2 gate.
Memory-bound: 32 MiB HBM traffic per core -> ~94 us floor at 358 GB/s.
"""

import numpy as np
import ml_dtypes

L = 8192
RANK = 8
BATCH = 4096
NCORES = 8
BS = BATCH // NCORES  # 512 batch columns per core
P = 128               # SBUF partitions
NCHUNK = L // P       # 64 row-chunks of 128
NB = 2                # batch halves per core (overlap stores_a with loads_b)
CB = BS // NB         # 256 columns per half
# Per-half load groups (in L-chunks). The SWDGE Q7 generates the next
# DMA's descriptors (~1.0 us fixed + 0.34 ns/desc) while the SDMA drains
# the current one, so each group's drain must cover the NEXT group's DGE
# time or the stream bubbles: an 8-chunk opener (drain 2.9 us > 16-chunk
# DGE 1.7 us) exposes 0.35 us less first-byte latency than a 16-chunk
# one, and the 4-chunk tail (drain 1.46 us) still covers the next half's
# 8-chunk DGE (1.34 us). Small tail groups also shrink the PE lag between
# last-load-end and the half's T completion.
XGROUPS = [8, 16, 16, 16, 4, 4]
# Per-half store stages (in L-chunks), geometric ramp: copies fill stages
# at ~0.165 us/chunk (ACT/DVE pairwise-concurrent) while stores drain at
# ~0.36 us/chunk, so the end-to-end bound is max over stages k of
# (copy time to finish stage k + drain time of stages k..end). Doubling
# stage sizes keeps every stage off that critical max; the tiny first
# stage starts the ring ~1 us after T.
YGROUPS = [2, 4, 8, 16, 16, 16, 2]
assert sum(XGROUPS) == NCHUNK
assert sum(YGROUPS) == NCHUNK  # 64 L-chunks of y per half

_NC = None  # cached compiled Bass module


def _body(tc, nc, x, vt, u, y, mybir):
    from contextlib import ExitStack

    f32 = mybir.dt.float32
    bf16 = mybir.dt.bfloat16
    x3 = x.rearrange("(n p) b -> p n b", p=P)   # [128, 64, 512] view of DRAM
    y3 = y.rearrange("(n p) b -> p n b", p=P)

    # Wait-topology discipline (walrus encodes ~1 sync wait per instruction):
    #  - load DMAs write distinct SBUF slots (no reuse waits);
    #  - the first PE instruction of each weight phase is a dummy matmul
    #    absorbing that weight tensor's DMA wait;
    #  - PSUM->SBUF copies alternate ACT/DVE (multi-sem waits legal there).
    with ExitStack() as ctx:
        const = ctx.enter_context(tc.tile_pool(name="const", bufs=1))
        # Slot count is per-tag: all max-size groups share tag "xtb", the
        # rest "xts", each with enough bufs that no slot is ever reused
        # (keeps the load DMAs wait-free).
        nbig = NB * sum(1 for g in XGROUPS if g == max(XGROUPS))
        nsmall = NB * len(XGROUPS) - nbig
        xpool = ctx.enter_context(tc.tile_pool(name="xbuf", bufs=1))
        tpsum = ctx.enter_context(tc.tile_pool(name="tpsum", bufs=1, space="PSUM"))
        ypsum = ctx.enter_context(tc.tile_pool(name="ypsum", bufs=4, space="PSUM"))
        ystage = ctx.enter_context(tc.tile_pool(name="ystage", bufs=4))

        # Issue ALL x load DMAs first (Pool-engine FIFO: they have no sync
        # waits, so the SWDGE ring streams both halves back to back while
        # the HWDGE store ring joins in as stages become ready).
        xts = []  # [half][group] -> SBUF tile
        for h in range(NB):
            tiles = []
            n0 = 0
            for g, xg in enumerate(XGROUPS):
                big = xg == max(XGROUPS)
                xt = xpool.tile([P, xg * CB], bf16,
                                tag="xtb" if big else "xts",
                                bufs=nbig if big else nsmall)
                nc.gpsimd.dma_start(
                    xt[:].rearrange("p (n b) -> p n b", b=CB),
                    x3[:, n0:n0 + xg, h * CB:(h + 1) * CB],
                )
                tiles.append(xt)
                n0 += xg
            xts.append(tiles)

        # Tiny replicated operands (already bf16 from the host), on the
        # HWDGE ring so they ride alongside the SWDGE x stream.
        vt_sb = const.tile([P, NCHUNK * RANK], bf16)  # vt[p, n*8+r] = V[r, n*128+p]
        nc.sync.dma_start(vt_sb[:], vt[:])
        u_sb = const.tile([RANK, L], bf16)
        nc.sync.dma_start(u_sb[:], u[:])

        # Dummy matmul reading ONLY vt_sb: absorbs the vt DMA wait so the
        # first real matmul carries a single sync wait.
        warm1 = tpsum.tile([RANK, RANK], f32, tag="warm1")
        nc.tensor.matmul(warm1[:], vt_sb[:, 0:RANK], vt_sb[:, 0:RANK],
                         start=True, stop=True)

        warmed2 = False
        for h in range(NB):
            # Phase 1 for this half: T_h = V @ x_h, accumulated in PSUM.
            t_ps = tpsum.tile([RANK, BS], f32, tag=f"t{h}")
            n0 = 0
            for g, xg in enumerate(XGROUPS):
                xt = xts[h][g]
                for c in range(xg):
                    n = n0 + c
                    nc.tensor.matmul(
                        t_ps[:, 0:CB],
                        vt_sb[:, n * RANK:(n + 1) * RANK],  # lhsT [128, 8] bf16
                        xt[:, c * CB:(c + 1) * CB],         # rhs  [128, 256] bf16
                        start=(n == 0),
                        stop=(n == NCHUNK - 1),
                    )
                n0 += xg
            # T to SBUF as bf16 (rhs of the expand matmuls), on DVE.
            t_sb = const.tile([RANK, CB], bf16, tag=f"tsb{h}")
            nc.vector.tensor_copy(t_sb[:], t_ps[:, 0:CB])

            if not warmed2:
                # Absorb the u DMA wait on a dummy matmul so the first y
                # matmul waits only on the t_sb copy.
                warm2 = tpsum.tile([P, RANK], f32, tag="warm2")
                nc.tensor.matmul(warm2[:], u_sb[:, 0:P], u_sb[:, 0:RANK],
                                 start=True, stop=True)
                warmed2 = True

            # Phase 2 for this half: y chunk = U_chunk^T @ T_h, staged
            # through SBUF, YGROUPS[d] chunks per HWDGE store.
            m0 = 0
            for d, yg in enumerate(YGROUPS):
                stage = ystage.tile([P, yg * CB], f32,
                                    tag=f"ys{yg}", bufs=3 if yg == 16 else 2)
                for c in range(yg):
                    n = m0 + c
                    y_ps = ypsum.tile([P, BS], f32, tag="yp")
                    nc.tensor.matmul(
                        y_ps[:, 0:CB],
                        u_sb[:, n * P:(n + 1) * P],  # lhsT [8, 128] bf16
                        t_sb[:],                     # rhs  [8, 256] bf16
                        start=True,
                        stop=True,
                    )
                    if c % 2 == 0:
                        nc.scalar.copy(stage[:, c * CB:(c + 1) * CB], y_ps[:, 0:CB])
                    else:
                        nc.vector.tensor_copy(stage[:, c * CB:(c + 1) * CB], y_ps[:, 0:CB])
                nc.sync.dma_start(
                    y3[:, m0:m0 + yg, h * CB:(h + 1) * CB],
                    stage[:].rearrange("p (n b) -> p n b", b=CB),
                )
                m0 += yg


def build_bass():
    import concourse.mybir as mybir
    import concourse.tile as tile
    from concourse import bacc

    # Bacc (not raw Bass): its compile() runs generate_event_semaphores(),
    # which splits multi-sem waits into the 1-wait-per-instruction form the
    # TRN2 ISA requires.
    nc = bacc.Bacc("TRN2", target_bir_lowering=False, debug=False)
    x = nc.dram_tensor("x", [L, BS], mybir.dt.float32, kind="ExternalInput").ap()
    vt = nc.dram_tensor("vt", [P, NCHUNK * RANK], mybir.dt.bfloat16, kind="ExternalInput").ap()
    u = nc.dram_tensor("u", [RANK, L], mybir.dt.bfloat16, kind="ExternalInput").ap()
    y = nc.dram_tensor("y", [L, BS], mybir.dt.float32, kind="ExternalOutput").ap()

    with tile.TileContext(nc) as tc:
        _body(tc, nc, x, vt, u, y, mybir)
    nc.compile()
    return nc


def _get_nc():
    global _NC
    if _NC is None:
        _NC = build_bass()
    return _NC


def make_in_maps(inputs, U, V):
    x = np.asarray(inputs, dtype=np.float32)
    U = np.asarray(U, dtype=np.float32)
    V = np.asarray(V, dtype=np.float32)
    u_bf = np.ascontiguousarray(U.astype(ml_dtypes.bfloat16))
    # vt[p, n*RANK + r] = V[r, n*128 + p]
    vt = np.ascontiguousarray(
        V.reshape(RANK, NCHUNK, P).transpose(2, 1, 0).reshape(P, NCHUNK * RANK)
        .astype(ml_dtypes.bfloat16)
    )
    in_maps = []
    for c in range(NCORES):
        xs = np.ascontiguousarray(x[:, c * BS:(c + 1) * BS])
        in_maps.append({"x": xs, "vt": vt, "u": u_bf})
    return in_maps


def kernel(inputs, U, V):
    from concourse import bass_utils

    nc = _get_nc()
    in_maps = make_in_maps(inputs, U, V)
    res = bass_utils.run_bass_kernel_spmd(nc, in_maps, core_ids=list(range(NCORES)))
    return np.concatenate([res.results[c]["y"] for c in range(NCORES)], axis=1)
